# revision 20
# baseline (speedup 1.0000x reference)
"""Single-head attention (B=8, S=2048, D=U=1024) on 8 TRN2 NeuronCores.

Sharding: data-parallel over batch - core b computes batch b end-to-end,
no cross-core communication.

Math restructure vs the straightforward Q/K path:
  scores = (x Wq + bq)(x Wk + bk)^T / 32
         = x A x^T  +  u 1^T  +  1 w^T  +  const,   A = Wq Wk^T / 32
  - u 1^T and the const shift scores per-QUERY only -> softmax-invariant,
    dropped exactly.
  - w_k = x_k . (Wk bq) / 32 varies per key: computed via tiny matvec
    chains and applied as the per-partition bias of the exp activation.
  - bv adds exactly bv to the output (probs sum to 1): folded into the
    final normalize epilogue.
  Score path: At = 2 Wk Wq^T (128 MMs), C = At^T xT (256), St = C^T xT
  (512) = 896 MMs instead of Q-proj + K-proj + scores = 1024, plus both
  bias epilogue passes disappear.

Layouts: the host passes xT [D,S], WqT/WkT [U,D], Wv [D,U] already
transposed and cast to fp16 (pure layout/dtype prep), so every SBUF load
is a straight strided DMA - no on-device transposes, no DRAM bounce.
Weights and xT halves are split across the sync and scalar HWDGE rings
so the first phase (At, weights only) can start ~3us in, behind a short
PE warmup burst that holds the HAM clock-gate open. Small per-partition
constants (mask columns, bq column) are derived on-chip via K=1
transpose matmuls instead of descriptor-storm gather DMAs.

Precision: fp16 storage everywhere (same PE throughput as bf16, 4x lower
quantization noise). The St matmul runs in fp8-e4m3 DoubleRow (2
weights/cell, K=256 per MM, 2x work per instruction): C is stored x64
(fp8-friendly scale) and exp() applies scale=1/64. ST_FP8_PAIRS in
{0..4} selects how many of the 4 dt-pair planes use fp8 (error dial:
4 -> rel_err ~1.77e-2, 3 -> ~1.53e-2, 0 -> ~4.9e-4).

Phase order (one long PE stream):
  warmup -> At -> V = xT^T Wv -> C -> consts/matvecs -> St(+mask,exp)
  -> PV + denominator -> normalize(+bv) -> out.
"""

import os
import sys

import numpy as np

for _p in ("/opt/trn_rl_repo", "/opt/pypackages"):
    if _p not in sys.path and os.path.isdir(_p):
        sys.path.append(_p)

import concourse.bass as bass
import concourse.tile as tile
from concourse import bacc, mybir
from concourse.bass import ts
from concourse.bass_utils import run_bass_kernel_spmd

P = 128
B, S, D, U = 8, 2048, 1024, 1024
NCORES = 8
NG = 512  # matmul moving free dim (one fp32 PSUM bank)
DT, UT, ST, KT = D // P, U // P, S // P, S // P  # 8, 8, 16, 16
QG = S // NG  # 4
UG = U // NG  # 2

# number of dt-pair planes of the St contraction done in fp8 DoubleRow
ST_FP8_PAIRS = 4  # 0..4

F32 = mybir.dt.float32
F16 = mybir.dt.float16
FP8 = mybir.dt.float8e4
I32 = mybir.dt.int32
AF = mybir.ActivationFunctionType
ALU = mybir.AluOpType
DR = mybir.MatmulPerfMode.DoubleRow

_cache = {}
last_results = None


def _emit(tc):
    nc = tc.nc
    xT_d = nc.dram_tensor("xt", [D, S], F16, kind="ExternalInput").ap()
    wqT_d = nc.dram_tensor("wqt", [U, D], F16, kind="ExternalInput").ap()
    wkT_d = nc.dram_tensor("wkt", [U, D], F16, kind="ExternalInput").ap()
    wv_d = nc.dram_tensor("wv", [D, U], F16, kind="ExternalInput").ap()
    m_d = nc.dram_tensor("mask", [1, S], I32, kind="ExternalInput").ap()
    bq_d = nc.dram_tensor("bq", [1, U], F32, kind="ExternalInput").ap()
    bv_d = nc.dram_tensor("bv", [1, U], F32, kind="ExternalInput").ap()
    out_d = nc.dram_tensor("out", [S, U], F32, kind="ExternalOutput").ap()

    NF8 = 2 * ST_FP8_PAIRS  # fp8 planes of the d-contraction
    with tc.tile_pool(name="big", bufs=1) as big:
        # ---------------- persistent tiles ----------------
        xT = big.tile([P, DT, S], F16, tag="xT", name="xT")
        wv_sb = big.tile([P, DT, U], F16, tag="wv", name="wv_sb")
        wqT_sb = big.tile([P, UT, D], F16, tag="wq", name="wqT_sb")
        wkT_sb = big.tile([P, UT, D], F16, tag="wk", name="wkT_sb")
        at_sb = big.tile([P, DT, D], F16, tag="at", name="at_sb")
        v_sb = big.tile([P, ST, U], F16, tag="v", name="v_sb")
        if NF8:
            c8_sb = big.tile([P, NF8, S], FP8, tag="c8", name="c8_sb")
            x8_sb = big.tile([P, NF8, S], FP8, tag="x8", name="x8_sb")
        if NF8 < DT:
            c16_sb = big.tile([P, DT - NF8, S], F16, tag="c16", name="c16_sb")
        # Et [k, q]: four 4-plane tiles; the first three reuse the wq/wk/at
        # slots (dead once At resp. C are done), the fourth is fresh.
        et_tiles = [
            big.tile([P, 4, S], F16, tag=t, name=f"et{i}")
            for i, t in enumerate(("wq", "wk", "at", "wv"))
        ]

        m_bcast = big.tile([P, S], FP8, tag="mb", name="m_bcast")  # 0/1 exact
        bv_bcast = big.tile([P, U], F32, tag="bvb", name="bv_bcast")
        consts = big.tile([P, 2 * KT], F32, tag="consts", name="consts")
        c_cols = consts[:, 0:KT]        # 64 * -10000 * (1-m_k)
        w_col = consts[:, KT : 2 * KT]  # x_k . (Wk bq) / 32
        cols16 = big.tile([P, 2 * DT + 2], F16, tag="c16s", name="cols16")
        bq_col = cols16[:, 0:DT]
        h_col = cols16[:, DT : 2 * DT]
        ones_col = cols16[:, 2 * DT : 2 * DT + 1]
        warm_w = big.tile([P, P], F16, tag="warmw", name="warm_w")
        rows = big.tile([1, S + 3 * U + P], F16, tag="rows", name="rows")
        m_row = rows[:, 0:S]
        h_row = rows[:, S : S + U]
        bv_row = rows[:, S + U : S + 2 * U]
        ones_row = rows[:, S + 2 * U : S + 2 * U + P]
        bq_row = rows[:, S + 2 * U + P : S + 3 * U + P]
        m_stage = big.tile([1, S], I32, tag="mst", name="m_stage")
        bqv_stage = big.tile([1, 2 * U], F32, tag="bqv", name="bqv_stage")
        bq_stage = bqv_stage[:, 0:U]
        bv_stage = bqv_stage[:, U : 2 * U]

        # ---------------- DMA loads ----------------
        # All on the two HWDGE rings (SWDGE/gpsimd is ~4x slower per byte).
        wq_src = wqT_d.rearrange("(t p) d -> p t d", p=P)
        wk_src = wkT_d.rearrange("(t p) d -> p t d", p=P)
        x_src = xT_d.rearrange("(t p) s -> p t s", p=P)
        # weights first and alone on both HWDGE rings (At's critical path),
        # in 2-plane chunks so the ut-outer At loop starts on chunk 0;
        # xT halves follow on both rings, wv trails on sync.
        for uc in range(0, UT, 2):
            nc.sync.dma_start(wqT_sb[:, uc : uc + 2, :], wq_src[:, uc : uc + 2, :])
            nc.scalar.dma_start(wkT_sb[:, uc : uc + 2, :], wk_src[:, uc : uc + 2, :])
        nc.sync.dma_start(xT[:, 0 : DT // 2, :], x_src[:, 0 : DT // 2, :])
        nc.scalar.dma_start(xT[:, DT // 2 : DT, :], x_src[:, DT // 2 : DT, :])
        nc.sync.dma_start(wv_sb[:], wv_d.rearrange("(t p) u -> p t u", p=P))
        nc.scalar.dma_start(m_stage[:], m_d)
        nc.scalar.dma_start(bq_stage, bq_d)
        nc.scalar.dma_start(bv_stage, bv_d)

        nc.vector.memset(warm_w[:], 1.0)
        nc.vector.memset(m_bcast[:], 0.0)
        nc.vector.memset(ones_row, 1.0)
        nc.vector.memset(ones_col, 1.0)

        # ---- PE warmup + At = 2 * Wk Wq^T  [d2, d1]  (2 = 64/32)
        # ut-outer with 4 d2-tiles x 2 dg = 8 PSUM banks per group, so the
        # accumulation paces with the 2-plane weight-chunk arrivals instead
        # of waiting for the full wqT/wkT.
        with tc.tile_pool(name="psAt", bufs=8, space="PSUM") as psA_pool:
            for i in range(16):
                pw = psA_pool.tile([P, NG], F32, tag="a", name=f"warm{i}")
                nc.tensor.matmul(
                    pw[:], lhsT=warm_w[:], rhs=m_bcast[:, 0:NG], start=True, stop=True
                )
            for d2g in range(2):
                psA = [
                    psA_pool.tile([P, NG], F32, tag="a", name="ps_at")
                    for _ in range(8)
                ]
                for ut in range(UT):
                    for i in range(4):
                        d2 = 4 * d2g + i
                        for dg in range(2):
                            nc.tensor.matmul(
                                psA[2 * i + dg][:],
                                lhsT=wkT_sb[:, ut, ts(d2, P)],
                                rhs=wqT_sb[:, ut, ts(dg, NG)],
                                start=(ut == 0),
                                stop=(ut == UT - 1),
                            )
                for i in range(4):
                    d2 = 4 * d2g + i
                    for dg in range(2):
                        nc.vector.tensor_scalar_mul(
                            at_sb[:, d2, ts(dg, NG)], psA[2 * i + dg][:], 2.0
                        )
            # early: unblock the later matvecs
            nc.vector.tensor_copy(m_row, m_stage[:])
            nc.vector.tensor_copy(bq_row, bq_stage)

        with tc.tile_pool(name="psMain", bufs=8, space="PSUM") as psM:
            # ---- V = xT^T Wv  [s, u]
            for ug in range(UG):
                for st in range(ST):
                    ps = psM.tile([P, NG], F32, tag="m", bufs=6, name="ps_v")
                    for dt in range(DT):
                        nc.tensor.matmul(
                            ps[:],
                            lhsT=xT[:, dt, ts(st, P)],
                            rhs=wv_sb[:, dt, ts(ug, NG)],
                            start=(dt == 0),
                            stop=(dt == DT - 1),
                        )
                    nc.vector.tensor_copy(v_sb[:, st, ts(ug, NG)], ps[:])

            # x8 = fp8 cast of the first NF8 planes of xT (DVE, overlaps C)
            for dt in range(NF8):
                nc.vector.tensor_copy(x8_sb[:, dt, :], xT[:, dt, :])

            # ---- C = At^T xT  [d1, k], stored x64-scaled (via the 2x in At)
            for kg in range(QG):
                for d1 in range(DT):
                    ps = psM.tile([P, NG], F32, tag="m", bufs=6, name="ps_c")
                    for d2 in range(DT):
                        nc.tensor.matmul(
                            ps[:],
                            lhsT=at_sb[:, d2, ts(d1, P)],
                            rhs=xT[:, d2, ts(kg, NG)],
                            start=(d2 == 0),
                            stop=(d2 == DT - 1),
                        )
                    if d1 < NF8:
                        nc.vector.tensor_copy(c8_sb[:, d1, ts(kg, NG)], ps[:])
                    else:
                        nc.vector.tensor_copy(c16_sb[:, d1 - NF8, ts(kg, NG)], ps[:])

            # ---- consts: mask cols, m/bv broadcasts, bias matvec chain.
            # Column layouts come from K=1 transpose matmuls on contiguous
            # rows (a strided gather DMA here costs tens of us in
            # descriptors).
            # Each batch writes disjoint columns of ONE psum tile (start only
            # on the group's first MM: later columns overwrite-where-unwritten),
            # so a single DVE op evacuates the whole batch.
            psk = psM.tile([P, KT], F32, tag="h", bufs=2, name="ps_mk")
            for kt in range(KT):
                nc.tensor.matmul(
                    psk[:, kt : kt + 1], lhsT=m_row[:, ts(kt, P)],
                    rhs=ones_row[:, 0:1],
                    start=(kt == 0), stop=(kt == KT - 1),
                )
            # c64 = m*640000 - 640000 -> 0 where m==1, -640000 where m==0
            nc.vector.tensor_scalar(
                c_cols, psk[:], 640000.0, -640000.0, ALU.mult, ALU.add
            )
            psq = psM.tile([P, UT], F32, tag="h", bufs=2, name="ps_bq")
            for ut in range(UT):
                nc.tensor.matmul(
                    psq[:, ut : ut + 1], lhsT=bq_row[:, ts(ut, P)],
                    rhs=ones_row[:, 0:1],
                    start=(ut == 0), stop=(ut == UT - 1),
                )
            nc.vector.tensor_copy(bq_col, psq[:])
            for dg in range(2):
                psh = psM.tile([1, NG], F32, tag="h", bufs=2, name="ps_h")
                for ut in range(UT):
                    nc.tensor.matmul(
                        psh[:],
                        lhsT=bq_col[:, ut : ut + 1],
                        rhs=wkT_sb[:, ut, ts(dg, NG)],
                        start=(ut == 0),
                        stop=(ut == UT - 1),
                    )
                nc.vector.tensor_copy(h_row[:, ts(dg, NG)], psh[:])
            psc = psM.tile([P, DT], F32, tag="h", bufs=2, name="ps_hc")
            for dt in range(DT):
                nc.tensor.matmul(
                    psc[:, dt : dt + 1], lhsT=h_row[:, ts(dt, P)],
                    rhs=ones_row[:, 0:1],
                    start=(dt == 0), stop=(dt == DT - 1),
                )
            nc.vector.tensor_copy(h_col, psc[:])
            psw = psM.tile([P, ST], F32, tag="h", bufs=2, name="ps_w")
            for st in range(ST):
                for dt in range(DT):
                    nc.tensor.matmul(
                        psw[:, st : st + 1],
                        lhsT=xT[:, dt, ts(st, P)],
                        rhs=h_col[:, dt : dt + 1],
                        start=(st == 0 and dt == 0),
                        stop=(st == ST - 1 and dt == DT - 1),
                    )
            nc.vector.tensor_scalar_mul(w_col, psw[:], 1.0 / 32.0)

            # m and bv broadcast across partitions via K=1 ones matmuls
            nc.vector.tensor_copy(bv_row, bv_stage)
            for qg in range(QG):
                psb = psM.tile([P, NG], F32, tag="m", bufs=6, name="ps_mb")
                nc.tensor.matmul(
                    psb[:], lhsT=ones_row[:, 0:P], rhs=m_row[:, ts(qg, NG)],
                    start=True, stop=True,
                )
                nc.vector.tensor_copy(m_bcast[:, ts(qg, NG)], psb[:])
            for ug in range(UG):
                psb = psM.tile([P, NG], F32, tag="m", bufs=6, name="ps_bvb")
                nc.tensor.matmul(
                    psb[:], lhsT=ones_row[:, 0:P], rhs=bv_row[:, ts(ug, NG)],
                    start=True, stop=True,
                )
                nc.vector.tensor_copy(bv_bcast[:, ts(ug, NG)], psb[:])

        # ---------------- St = C^T xT (x64) + mask -> exp ----------------
        with tc.tile_pool(name="psSt", bufs=6, space="PSUM") as psS:
            for kt in range(KT):
                pss = [
                    psS.tile([P, NG], F32, tag="s", name="ps_st") for _ in range(QG)
                ]
                for j in range(ST_FP8_PAIRS):
                    for qg in range(QG):
                        nc.tensor.matmul(
                            pss[qg][:],
                            lhsT=c8_sb[:, 2 * j : 2 * j + 2, ts(kt, P)],
                            rhs=x8_sb[:, 2 * j : 2 * j + 2, ts(qg, NG)],
                            perf_mode=DR,
                            start=(j == 0),
                            stop=(NF8 == DT and j == ST_FP8_PAIRS - 1),
                        )
                for dt in range(NF8, DT):
                    for qg in range(QG):
                        nc.tensor.matmul(
                            pss[qg][:],
                            lhsT=c16_sb[:, dt - NF8, ts(kt, P)],
                            rhs=xT[:, dt, ts(qg, NG)],
                            start=(dt == 0),
                            stop=(dt == DT - 1),
                        )
                for qg in range(QG):
                    # scores64 += m_q * c64_k   (rank-1 mask, on DVE)
                    nc.vector.scalar_tensor_tensor(
                        pss[qg][:],
                        m_bcast[:, ts(qg, NG)],
                        c_cols[:, kt : kt + 1],
                        pss[qg][:],
                        ALU.mult,
                        ALU.add,
                    )
                    nc.scalar.activation(
                        et_tiles[kt // 4][:, kt % 4, ts(qg, NG)],
                        pss[qg][:],
                        AF.Exp,
                        bias=w_col[:, kt : kt + 1],
                        scale=1.0 / 64.0,
                    )

        # ---------------- PV + denominator + normalize(+bv) ----------------
        with tc.tile_pool(name="psPV", bufs=8, space="PSUM") as psE:
            for qt in range(KT):
                pc = [
                    psE.tile([P, NG], F32, tag="ctx", bufs=4, name="ps_ctx")
                    for _ in range(UG)
                ]
                den = psE.tile([P, 1], F32, tag="den", bufs=2, name="ps_den")
                for kt in range(KT):
                    lhsT = et_tiles[kt // 4][:, kt % 4, ts(qt, P)]
                    first, last = kt == 0, kt == KT - 1
                    for ug in range(UG):
                        nc.tensor.matmul(
                            pc[ug][:],
                            lhsT=lhsT,
                            rhs=v_sb[:, kt, ts(ug, NG)],
                            start=first,
                            stop=last,
                        )
                    nc.tensor.matmul(
                        den[:], lhsT=lhsT, rhs=ones_col, start=first, stop=last
                    )
                recip = big.tile([P, 1], F32, tag="recip", bufs=2, name="recip")
                nc.vector.reciprocal(recip[:], den[:])
                o = big.tile([P, U], F32, tag="o", bufs=2, name="o_sb")
                for ug in range(UG):
                    # out = ctx * (1/den) + bv
                    nc.vector.scalar_tensor_tensor(
                        o[:, ts(ug, NG)],
                        pc[ug][:],
                        recip[:],
                        bv_bcast[:, ts(ug, NG)],
                        ALU.mult,
                        ALU.add,
                    )
                nc.sync.dma_start(out_d[ts(qt, P), :], o[:])


def _build():
    if "nc" in _cache:
        return _cache["nc"]
    nc = bacc.Bacc("TRN2", target_bir_lowering=False, debug=False, num_devices=NCORES)
    with tile.TileContext(nc) as tc:
        _emit(tc)
    nc.compile()
    _cache["nc"] = nc
    return nc


def kernel(x, mask, Wq, bq, Wk, bk, Wv, bv):
    global last_results
    nc = _build()
    wqT = np.ascontiguousarray(np.asarray(Wq, np.float32).T).astype(np.float16)
    wkT = np.ascontiguousarray(np.asarray(Wk, np.float32).T).astype(np.float16)
    wv = np.ascontiguousarray(Wv, dtype=np.float32).astype(np.float16)
    bqr = np.ascontiguousarray(bq, dtype=np.float32).reshape(1, U)
    bvr = np.ascontiguousarray(bv, dtype=np.float32).reshape(1, U)
    _ = bk  # shifts scores per-query only: softmax-invariant, drops out exactly
    in_maps = []
    for b in range(B):
        in_maps.append(
            {
                "xt": np.ascontiguousarray(np.asarray(x[b], np.float32).T).astype(
                    np.float16
                ),
                "mask": np.ascontiguousarray(mask[b], dtype=np.int32).reshape(1, S),
                "wqt": wqT,
                "wkt": wkT,
                "wv": wv,
                "bq": bqr,
                "bv": bvr,
            }
        )
    res = run_bass_kernel_spmd(
        nc,
        in_maps,
        core_ids=list(range(NCORES)),
        trace=bool(int(os.environ.get("KERNEL_TRACE", "0"))),
        tmpdir=os.environ.get("KERNEL_TRACE_DIR"),
    )
    last_results = res
    return np.stack([res.results[b]["out"] for b in range(B)])
